# revision 1
# baseline (speedup 1.0000x reference)
"""Distributed attention kernel for Trainium2 (8 NeuronCores).

Module: x @ w_qkv -> per-head softmax(q k^T / sqrt(hd)) @ v -> out @ w_proj + b.
Shapes: B=2, N=2048, DIM=1024, H=16, HD=64, f32 in/out; bf16 matmul compute
(f32 PSUM accumulation), rel err ~5e-3 vs the f32 reference.

Sharding: core i handles batch b=i//4 and head-group g=i%4 (4 heads).
- qT/kT computed feature-major [256, 2048]; v token-major with a fused
  ones column so the AV matmul also produces softmax denominators.
- Attention per head pair (2j, 2j+1): S^T matmuls go to PE row-groups 0
  and 64 (concurrent on HW), packed side by side in one PSUM tile so a
  single Exp covers both heads; exp on ScalarE straight from PSUM
  (logits are O(1): no max subtraction needed). AV (V stationary,
  producing O^T) is pipelined one strip behind S/exp so ScalarE never
  starves. The j=1 QKV matmuls and the V projection are emitted as PE
  gap-fillers inside pair 0's strips.
- One AllGather per head pair across the 4 cores of a batch group
  (issued as soon as that pair finishes), then each core computes a
  256-column slice of the projection (+bias via a rank-1 K=1 matmul),
  accumulating gather-0 k-tiles first.
Host assembles the 8 per-core [2048, 256] outputs into [2, 2048, 1024].
"""

import sys, os

for _p in ("/opt/trn_rl_repo", "/opt/pypackages"):
    if _p not in sys.path:
        sys.path.insert(0, _p)

import numpy as np
import ml_dtypes
from contextlib import ExitStack

import concourse.bass as bass
import concourse.bacc as bacc
import concourse.mybir as mybir
from concourse import tile
from concourse.bass_utils import run_bass_kernel_spmd

F32 = mybir.dt.float32
BF16 = mybir.dt.bfloat16
NPBF16 = np.dtype(ml_dtypes.bfloat16)

P = 128
NTOK = 2048
C = 1024
NH = 4          # heads per core
HD = 64
FEAT = NH * HD  # 256
KT = C // P     # 8 contraction tiles for qkv
MT = NTOK // P  # 16 token tiles
SCALE = HD ** -0.5
N_CORES = 8
GROUPS = [[0, 1, 2, 3], [4, 5, 6, 7]]

AF = mybir.ActivationFunctionType


def build_program(nc):
    xT = nc.dram_tensor("xT", [C, NTOK], BF16, kind="ExternalInput").ap()
    wq = nc.dram_tensor("wq", [C, FEAT], BF16, kind="ExternalInput").ap()
    wk = nc.dram_tensor("wk", [C, FEAT], BF16, kind="ExternalInput").ap()
    wv = nc.dram_tensor("wv", [C, FEAT], BF16, kind="ExternalInput").ap()
    wp = nc.dram_tensor("wp", [C, FEAT], BF16, kind="ExternalInput").ap()
    bp = nc.dram_tensor("bp", [1, FEAT], BF16, kind="ExternalInput").ap()
    ones_in = nc.dram_tensor("ones", [1, P], BF16, kind="ExternalInput").ap()
    out_e = nc.dram_tensor("out", [NTOK, FEAT], F32, kind="ExternalOutput").ap()

    # Internal DRAM for the collectives (one AllGather per head pair).
    cc_in = [nc.dram_tensor(f"cc_in{j}", [P, NTOK], BF16) for j in range(2)]
    cc_out = [nc.dram_tensor(f"cc_out{j}", [4 * P, NTOK], BF16)
              for j in range(2)]

    with tile.TileContext(nc) as tc, ExitStack() as ctx:
        persist = ctx.enter_context(tc.tile_pool(name="persist", bufs=1))
        psum = ctx.enter_context(tc.tile_pool(name="psum", bufs=1, space="PSUM"))

        # ---- constants ----
        ones_row = persist.tile([1, P], BF16, tag="ones_row")
        nc.sync.dma_start(ones_row[:], ones_in[:])
        ones64 = ones_row[:, 0:64]
        bias_sb = persist.tile([1, FEAT], BF16, tag="bias")
        nc.sync.dma_start(bias_sb[:], bp[:])

        # ---- persistent activations ----
        qT = [persist.tile([P, NTOK], BF16, tag=f"qT{j}", name=f"qT{j}")
              for j in range(2)]
        kT = [persist.tile([P, NTOK], BF16, tag=f"kT{j}", name=f"kT{j}")
              for j in range(2)]
        # v token-major, per head 64 features + a ones column (65 each)
        v_sb = [persist.tile([P, NH * 65], BF16, tag=f"v{m}", name=f"v{m}")
                for m in range(MT)]
        oT = [persist.tile([P, NTOK], BF16, tag=f"oT{j}", name=f"oT{j}")
              for j in range(2)]

        pa = ctx.enter_context(tc.tile_pool(name="stage_a", bufs=1))
        pb = ctx.enter_context(tc.tile_pool(name="stage_b", bufs=1))

        # ---- input loads: weights first (small), x split in halves ----
        x_sb = [pa.tile([P, NTOK], BF16, tag=f"x{k}", name=f"x{k}")
                for k in range(KT)]
        w_sb = {}
        for name, ap in (("wq", wq), ("wk", wk), ("wv", wv)):
            w_sb[name] = [pa.tile([P, FEAT], BF16, tag=f"{name}{k}",
                                  name=f"{name}{k}") for k in range(KT)]
        # queue order = need order: wk + x first halves feed the first
        # qkv chunk; wq/wv and the second halves can trail
        for k in range(KT):
            nc.sync.dma_start(w_sb["wk"][k][:], wk[k * P:(k + 1) * P, :])
        for k in range(KT):
            nc.sync.dma_start(x_sb[k][:, 0:NTOK // 2],
                              xT[k * P:(k + 1) * P, 0:NTOK // 2])
        for k in range(KT):
            nc.sync.dma_start(x_sb[k][:, NTOK // 2:],
                              xT[k * P:(k + 1) * P, NTOK // 2:])
        for k in range(KT):
            nc.sync.dma_start(w_sb["wq"][k][:], wq[k * P:(k + 1) * P, :])
        for k in range(KT):
            nc.sync.dma_start(w_sb["wv"][k][:], wv[k * P:(k + 1) * P, :])

        # ---- stage A emitters (interleaved into attention as gap fill) ----
        def emit_qk(j, names=("wq", "wk"), chunks=(0, 1, 2, 3)):
            for wname in names:
                dst = qT if wname == "wq" else kT
                for s in chunks:
                    ps = psum.tile([P, 512], F32, tag="ps_a", bufs=3,
                                   name="ps_qk")
                    for k in range(KT):
                        nc.tensor.matmul(
                            ps[:],
                            lhsT=w_sb[wname][k][:, j * P:(j + 1) * P],
                            rhs=x_sb[k][:, s * 512:(s + 1) * 512],
                            start=(k == 0), stop=(k == KT - 1),
                        )
                    nc.vector.tensor_copy(dst[j][:, s * 512:(s + 1) * 512],
                                          ps[:])

        def emit_v(lo=0, hi=MT):
            for m in range(lo, hi):
                ps = psum.tile([P, FEAT], F32, tag="ps_a", bufs=3,
                               padded_shape=[P, 512], name="ps_v")
                for k in range(KT):
                    nc.tensor.matmul(
                        ps[:],
                        lhsT=x_sb[k][:, m * P:(m + 1) * P],
                        rhs=w_sb["wv"][k][:],
                        start=(k == 0), stop=(k == KT - 1),
                    )
                nc.gpsimd.memset(v_sb[m][:], 1.0)
                dst = v_sb[m][:].rearrange("p (h e) -> p h e", e=65)[:, :, 0:64]
                src = ps[:].rearrange("p (h e) -> p h e", e=64)
                nc.vector.tensor_copy(dst, src)

        # ---- stage B: attention, AV pipelined one strip behind S/exp ----
        def emit_av_norm(j, s, pt_tiles):
            m0 = s * 512
            for i in range(2):
                h, po = 2 * j + i, i * 64
                ps_o = psum.tile([65, 512], F32, tag="ps_a", bufs=3,
                                 padded_shape=[P, 512], name="ps_o")
                for n in range(MT):
                    nc.tensor.matmul(
                        ps_o[:],
                        lhsT=v_sb[n][:, h * 65:(h + 1) * 65],
                        rhs=pt_tiles[n][:, i * 512:(i + 1) * 512],
                        start=(n == 0), stop=(n == MT - 1),
                    )
                # evacuate PSUM right away so the next AV can reuse the
                # slot; the whole normalize chain then runs off-PSUM
                o65 = pb.tile([65, 512], F32, tag="o65", bufs=2)
                nc.vector.tensor_copy(o65[:], ps_o[:])
                # normalize: row 64 is the softmax denominator
                rec = pb.tile([1, 512], BF16, tag="rec", bufs=2)
                with nc.allow_low_precision(reason="bf16 recip"):
                    nc.vector.reciprocal(rec[:], o65[64:65, :])
                ps_b = psum.tile([64, 512], F32, tag="ps_b", bufs=1)
                nc.tensor.matmul(ps_b[:], lhsT=ones64, rhs=rec[:],
                                 start=True, stop=True)
                bc_sb = pb.tile([64, 512], F32, tag="bc", bufs=2)
                nc.vector.tensor_copy(bc_sb[:], ps_b[:])
                nc.vector.tensor_mul(oT[j][po:po + 64, m0:m0 + 512],
                                     o65[0:64, :], bc_sb[:])

        def emit_gather(j):
            half = NTOK // 2
            nc.sync.dma_start(cc_in[j][:, 0:half], oT[j][:, 0:half])
            nc.sync.dma_start(cc_in[j][:, half:], oT[j][:, half:])
            if os.environ.get("KMODE") == "nocc":
                for g in range(4):
                    nc.gpsimd.dma_start(cc_out[j][g * P:(g + 1) * P, :],
                                        cc_in[j][:, :])
            else:
                nc.gpsimd.collective_compute(
                    "AllGather",
                    mybir.AluOpType.bypass,
                    ins=[cc_in[j][:, :]],
                    outs=[cc_out[j][:, :]],
                    replica_groups=GROUPS,
                )

        def emit_s_exp(j, s):
            m0 = s * 512
            pt_tiles = []
            for n in range(MT):
                ps_s = psum.tile([P, 1024], F32, tag="ps_s", bufs=2)
                for i in range(2):      # head 2j at cols 0:512, 2j+1 after
                    po = i * 64
                    nc.tensor.matmul(
                        ps_s[:, i * 512:(i + 1) * 512],
                        lhsT=kT[j][po:po + 64, n * P:(n + 1) * P],
                        rhs=qT[j][po:po + 64, m0:m0 + 512],
                        start=True, stop=True,
                    )
                pt = pb.tile([P, 1024], BF16, tag="pt", bufs=34)
                nc.scalar.activation(pt[:], ps_s[:], AF.Exp, scale=SCALE)
                pt_tiles.append(pt)
            return pt_tiles

        # one flat pipeline over the 8 (pair, strip) steps; AV runs one
        # strip behind S/exp so ScalarE never waits at strip boundaries.
        # Strip 0 needs all of kT0 but only the first qT0 chunk, so emit
        # just those before attention starts; the rest fills PE gaps.
        emit_qk(0, names=("wk",))
        emit_qk(0, names=("wq",), chunks=(0,))
        # gap-fill balanced against the exp pace (~18.4us per strip):
        # v must be fully emitted before the first AV (PE is strict FIFO,
        # a later v matmul would deadlock an earlier AV that reads it);
        # qT1 chunk s is only needed by pair-1 strip s, so qk(1) spreads
        # deep into pair 1's hook slots.
        hooks = {
            0: lambda: (emit_qk(0, names=("wq",), chunks=(1,)), emit_v(0, 11)),
            1: lambda: (emit_qk(0, names=("wq",), chunks=(2,)), emit_v(11, MT)),
            2: lambda: (emit_qk(0, names=("wq",), chunks=(3,)),
                        emit_qk(1, names=("wk",), chunks=(0, 1))),
            3: lambda: (emit_qk(1, names=("wk",), chunks=(2, 3)),
                        emit_qk(1, names=("wq",), chunks=(0,))),
            4: lambda: emit_qk(1, names=("wq",), chunks=(1,)),
            5: lambda: emit_qk(1, names=("wq",), chunks=(2,)),
            6: lambda: emit_qk(1, names=("wq",), chunks=(3,)),
        }
        strips = [(j, s) for j in range(2) for s in range(4)]
        pending = None
        for gi, (j, s) in enumerate(strips):
            pt_tiles = emit_s_exp(j, s)
            if gi in hooks:
                hooks[gi]()             # PE gap fill under the exp stream
            if pending is not None:
                emit_av_norm(pending[0], pending[1], pending[2])
                pending = None
            if s == 3:
                # drain the pair's last AV right away so its AllGather
                # launches as soon as the exps finish (the serialized
                # gather chain is the critical tail)
                emit_av_norm(j, s, pt_tiles)
                emit_gather(j)
            else:
                pending = (j, s, pt_tiles)

        # scheduler-only fence: keep stage D's PE work out of the attention
        # stream (PE is strict FIFO; an early proj ldweights waiting on the
        # gather would block everything behind it)
        tc.no_sync_barrier()

        # ---- stage D: projection column slice, two passes ----
        # pass 1 (after gather 0): accumulate even k-tiles into SBUF;
        # pass 2 (after gather 1): odd k-tiles + bias, add pass-1 partials.
        with tc.tile_pool(name="stage_d", bufs=1) as pd:
            # reuse the x slots (same shape/dtype, long dead by now)
            ot_full = [pa.tile([P, NTOK], BF16, tag=f"x{k}", name=f"of{k}")
                       for k in range(KT)]
            wp_sb = [pd.tile([P, FEAT], BF16, tag=f"wp{k}", name=f"wp{k}")
                     for k in range(KT)]
            for k in range(KT):
                nc.sync.dma_start(wp_sb[k][:], wp[k * P:(k + 1) * P, :])
            K_ORDER = [0, 2, 4, 6, 1, 3, 5, 7]  # pair-0 gather lands first
            for k in K_ORDER:       # halves spread the queue load
                half = NTOK // 2
                src_ap = cc_out[k % 2][(k // 2) * P:(k // 2 + 1) * P, :]
                nc.sync.dma_start(ot_full[k][:, 0:half], src_ap[:, 0:half])
                nc.sync.dma_start(ot_full[k][:, half:], src_ap[:, half:])

            acc = [pd.tile([P, FEAT], F32, tag=f"acc{m}", name=f"acc{m}")
                   for m in range(MT)]
            for m in range(MT):
                ps = psum.tile([P, FEAT], F32, tag="ps_a", bufs=3,
                               padded_shape=[P, 512], name="ps_proj1")
                for ki, k in enumerate(K_ORDER[:4]):
                    nc.tensor.matmul(
                        ps[:],
                        lhsT=ot_full[k][:, m * P:(m + 1) * P],
                        rhs=wp_sb[k][:],
                        start=(ki == 0), stop=(ki == 3),
                    )
                nc.vector.tensor_copy(acc[m][:], ps[:])
            for m in range(MT):
                ps = psum.tile([P, FEAT], F32, tag="ps_a", bufs=3,
                               padded_shape=[P, 512], name="ps_proj2")
                for ki, k in enumerate(K_ORDER[4:]):
                    nc.tensor.matmul(
                        ps[:],
                        lhsT=ot_full[k][:, m * P:(m + 1) * P],
                        rhs=wp_sb[k][:],
                        start=(ki == 0), stop=False,
                    )
                nc.tensor.matmul(ps[:], lhsT=ones_row[:], rhs=bias_sb[:],
                                 start=False, stop=True)
                o_sb = pd.tile([P, FEAT], F32, tag="osb", bufs=3)
                nc.vector.tensor_add(o_sb[:], ps[:], acc[m][:])
                nc.sync.dma_start(out_e[m * P:(m + 1) * P, :], o_sb[:])

    return nc


_CACHE = {}


def _get_nc():
    if "nc" not in _CACHE:
        nc = bacc.Bacc("TRN2", target_bir_lowering=False, debug=False,
                       num_devices=N_CORES)
        nc = build_program(nc)
        nc.compile()
        _CACHE["nc"] = nc
    return _CACHE["nc"]


def make_in_maps(x, w_qkv, w_proj, b_proj):
    in_maps = []
    for core in range(N_CORES):
        b, g = core // 4, core % 4
        hs = slice(g * FEAT, (g + 1) * FEAT)
        in_maps.append({
            "xT": np.ascontiguousarray(x[b].T).astype(NPBF16),
            "wq": np.ascontiguousarray(w_qkv[:, 0:1024][:, hs]).astype(NPBF16),
            "wk": np.ascontiguousarray(w_qkv[:, 1024:2048][:, hs]).astype(NPBF16),
            "wv": np.ascontiguousarray(w_qkv[:, 2048:3072][:, hs]).astype(NPBF16),
            "wp": np.ascontiguousarray(w_proj[:, hs]).astype(NPBF16),
            "bp": np.ascontiguousarray(b_proj[hs]).reshape(1, FEAT).astype(NPBF16),
            "ones": np.ones((1, P), NPBF16),
        })
    return in_maps


def assemble(results):
    out = np.empty((2, NTOK, 1024), np.float32)
    for core in range(N_CORES):
        b, g = core // 4, core % 4
        out[b][:, g * FEAT:(g + 1) * FEAT] = results[core]["out"]
    return out


def kernel(x, w_qkv, w_proj, b_proj, trace=False):
    nc = _get_nc()
    in_maps = make_in_maps(np.asarray(x), np.asarray(w_qkv),
                           np.asarray(w_proj), np.asarray(b_proj))
    res = run_bass_kernel_spmd(nc, in_maps, core_ids=list(range(N_CORES)),
                               trace=trace)
    out = assemble(res.results)
    if trace:
        return out, res
    return out



# revision 31
# speedup vs baseline: 1.1355x; 1.1355x over previous
"""Distributed attention kernel for Trainium2 (8 NeuronCores).

Module: x @ w_qkv -> per-head softmax(q k^T / sqrt(hd)) @ v -> out @ w_proj + b.
Shapes: B=2, N=2048, DIM=1024, H=16, HD=64, f32 in/out; bf16 matmul compute
(f32 PSUM accumulation).

Sharding: core i handles batch b=i//4 and head-group g=i%4 (4 heads).
- qT/kT computed feature-major [256, 2048]; v token-major with a fused
  ones column per head (65 cols) so the AV matmul also produces softmax
  denominators.
- Attention strips alternate head pairs ((0,s) then (1,s) per query
  strip s) so both pairs' outputs for a token range complete together.
  S^T matmuls for the two heads of a pair pack side by side in one
  [128, 1024] PSUM tile so a single Exp covers both heads (logits are
  O(1): no max subtraction needed).
- AV is P-stationary: out = O [128 queries, 65] per (head, q-chunk), so
  each accumulation chunk costs only 65 output rows instead of 512.  AV
  chunks pipeline one strip behind S/exp (chunk n of strip t interleaves
  after S tile n+2 of strip t+1) so PE chases the exp stream.
- Softmax normalize: row 64 of O is the denominator -> DVE reciprocal +
  per-partition tensor_scalar_mul; the token-major [128, 128] (2-head)
  result flips to feature-major oT via DMA transpose (xbar).
- Exchange: per token quarter, each core computes its partial projection
  y^T[1024 out-cols, 512 tokens] (contraction over its own 256 features
  only) and a ReduceScatter over the 4-core batch group sums partials
  and leaves each core its 256 out-columns.  Quarters 0-2 overlap the
  remaining attention strips; only quarter 3's RS is on the tail.
- Bias is a per-partition scalar add on the feature-major result.
Host assembles the 8 per-core [256, 2048] outputs (transposed) into
[2, 2048, 1024].
"""

import sys, os

for _p in ("/opt/trn_rl_repo", "/opt/pypackages"):
    if _p not in sys.path:
        sys.path.insert(0, _p)

import numpy as np
import ml_dtypes
from contextlib import ExitStack

import concourse.bass as bass
import concourse.bacc as bacc
import concourse.mybir as mybir
from concourse import tile
from concourse.bass_utils import run_bass_kernel_spmd

F32 = mybir.dt.float32
BF16 = mybir.dt.bfloat16
NPBF16 = np.dtype(ml_dtypes.bfloat16)

P = 128
NTOK = 2048
C = 1024
NH = 4          # heads per core
HD = 64
FEAT = NH * HD  # 256
KT = C // P     # 8 contraction tiles for qkv
MT = NTOK // P  # 16 key tiles per strip
QS = 512        # queries per strip
SCALE = HD ** -0.5
N_CORES = 8
GROUPS = [[0, 1, 2, 3], [4, 5, 6, 7]]

AF = mybir.ActivationFunctionType


def build_program(nc):
    xT = nc.dram_tensor("xT", [C, NTOK], BF16, kind="ExternalInput").ap()
    wq = nc.dram_tensor("wq", [C, FEAT], BF16, kind="ExternalInput").ap()
    wk = nc.dram_tensor("wk", [C, FEAT], BF16, kind="ExternalInput").ap()
    wv = nc.dram_tensor("wv", [C, FEAT], BF16, kind="ExternalInput").ap()
    # w_proj ROW slice for this core's features: [FEAT, C]
    wpr = nc.dram_tensor("wpr", [FEAT, C], BF16, kind="ExternalInput").ap()
    # full projection bias (pre-scaled by 1/4 on host: the ReduceScatter
    # sums it back across the 4-core group), added into the partials
    bpf = nc.dram_tensor("bpf", [C, 1], F32, kind="ExternalInput").ap()
    ident = nc.dram_tensor("ident", [P, P], BF16, kind="ExternalInput").ap()
    # out is per-quarter feature-major [4, FEAT, QS] bf16, each quarter a
    # contiguous block written directly by its ReduceScatter; the host
    # reassembles and upcasts
    out_e = nc.dram_tensor("out", [4, FEAT, QS], BF16, kind="ExternalOutput").ap()

    # Internal DRAM for the per-quarter ReduceScatter.
    cc_in = [nc.dram_tensor(f"cc_in{q}", [C, QS], BF16) for q in range(4)]
    cc_out = [nc.dram_tensor(f"cc_out{q}", [FEAT, QS], BF16) for q in range(4)]

    with tile.TileContext(nc) as tc, ExitStack() as ctx:
        persist = ctx.enter_context(tc.tile_pool(name="persist", bufs=1))
        psum = ctx.enter_context(tc.tile_pool(name="psum", bufs=1, space="PSUM"))

        bias_sb = [persist.tile([P, 1], F32, tag=f"bias{i}", name=f"bias{i}")
                   for i in range(KT)]
        for i in range(KT):
            nc.sync.dma_start(bias_sb[i][:], bpf[i * P:(i + 1) * P, :])
        ident_sb = persist.tile([P, P], BF16, tag="ident")
        nc.sync.dma_start(ident_sb[:], ident[:])

        # ---- persistent activations ----
        qT = [persist.tile([P, NTOK], BF16, tag=f"qT{j}", name=f"qT{j}")
              for j in range(2)]
        kT = [persist.tile([P, NTOK], BF16, tag=f"kT{j}", name=f"kT{j}")
              for j in range(2)]
        v_sb = [persist.tile([P, NH * 65], BF16, tag=f"v{m}", name=f"v{m}")
                for m in range(MT)]
        oT = [persist.tile([P, NTOK], BF16, tag=f"oT{j}", name=f"oT{j}")
              for j in range(2)]

        pa = ctx.enter_context(tc.tile_pool(name="stage_a", bufs=1))
        pb = ctx.enter_context(tc.tile_pool(name="stage_b", bufs=1))

        # ---- input loads ----
        x_sb = [pa.tile([P, NTOK], BF16, tag=f"x{k}", name=f"x{k}")
                for k in range(KT)]
        w_sb = {}
        for name, ap in (("wq", wq), ("wk", wk), ("wv", wv)):
            w_sb[name] = [pa.tile([P, FEAT], BF16, tag=f"{name}{k}",
                                  name=f"{name}{k}") for k in range(KT)]
        wpr_sb = [pa.tile([P, C], BF16, tag=f"wpr{j}", name=f"wpr{j}")
                  for j in range(2)]
        # x in column-quarters: the first S tile only needs quarter 0, so
        # interleave weight loads between quarters in need order.
        def load_xq(q):
            for k in range(KT):
                nc.sync.dma_start(x_sb[k][:, q * 512:(q + 1) * 512],
                                  xT[k * P:(k + 1) * P, q * 512:(q + 1) * 512])

        load_xq(0)
        for k in range(KT):
            nc.sync.dma_start(w_sb["wk"][k][:], wk[k * P:(k + 1) * P, :])
        for k in range(KT):
            nc.sync.dma_start(w_sb["wq"][k][:], wq[k * P:(k + 1) * P, :])
        load_xq(1)
        for k in range(KT):
            nc.sync.dma_start(w_sb["wv"][k][:], wv[k * P:(k + 1) * P, :])
        load_xq(2)
        load_xq(3)
        for j in range(2):
            nc.sync.dma_start(wpr_sb[j][:], wpr[j * P:(j + 1) * P, :])

        # ---- qkv emitters (PE gap fill) ----
        def emit_qk(j, names=("wq", "wk"), chunks=(0, 1, 2, 3)):
            for wname in names:
                dst = qT if wname == "wq" else kT
                for s in chunks:
                    ps = psum.tile([P, 512], F32, tag="ps_a", bufs=2,
                                   name="ps_qk")
                    for k in range(KT):
                        nc.tensor.matmul(
                            ps[:],
                            lhsT=w_sb[wname][k][:, j * P:(j + 1) * P],
                            rhs=x_sb[k][:, s * 512:(s + 1) * 512],
                            start=(k == 0), stop=(k == KT - 1),
                        )
                    nc.vector.tensor_copy(dst[j][:, s * 512:(s + 1) * 512],
                                          ps[:])

        def emit_v(lo=0, hi=MT):
            for m in range(lo, hi):
                ps = psum.tile([P, FEAT], F32, tag="ps_a", bufs=2,
                               padded_shape=[P, 512], name="ps_v")
                for k in range(KT):
                    nc.tensor.matmul(
                        ps[:],
                        lhsT=x_sb[k][:, m * P:(m + 1) * P],
                        rhs=w_sb["wv"][k][:],
                        start=(k == 0), stop=(k == KT - 1),
                    )
                nc.gpsimd.memset(v_sb[m][:], 1.0)
                dst = v_sb[m][:].rearrange("p (h e) -> p h e", e=65)[:, :, 0:64]
                src = ps[:].rearrange("p (h e) -> p h e", e=64)
                nc.vector.tensor_copy(dst, src)

        # ---- S + exp for one (strip, key-tile) ----
        def emit_s_tile(j, s, n):
            ps = psum.tile([P, 1024], F32, tag="ps_s", bufs=2, name="ps_s")
            for i in range(2):      # head 2j at cols 0:512, 2j+1 after
                po = i * 64
                nc.tensor.matmul(
                    ps[:, i * 512:(i + 1) * 512],
                    lhsT=kT[j][po:po + 64, n * P:(n + 1) * P],
                    rhs=qT[j][po:po + 64, s * 512:(s + 1) * 512],
                    start=True, stop=True,
                )
            pt = pb.tile([P, 1024], BF16, tag="pt", bufs=20, name="pt")
            nc.scalar.activation(pt[:], ps[:], AF.Exp, scale=SCALE)
            return pt

        # ---- AV chunk: P-stationary, out O [128 q, 65] per (head, qc) ----
        def emit_av_chunk(st, n):
            j, pts = st["j"], st["pt"]
            if st["po"] is None:
                st["po"] = [psum.tile([P, 512], F32, tag="ps_o", bufs=2,
                                      name=f"po{j}{st['s']}{i}")
                            for i in range(2)]
            # one accumulation group per head's PSUM bank: only the very
            # first matmul starts it, only the very last stops it (PSUM
            # allows a single pending group per zero region)
            for i in range(2):
                h = 2 * j + i
                for qc in range(4):
                    nc.tensor.matmul(
                        st["po"][i][:, qc * 65:qc * 65 + 65],
                        lhsT=pts[n][:, i * 512 + qc * P:i * 512 + (qc + 1) * P],
                        rhs=v_sb[n][:, h * 65:(h + 1) * 65],
                        start=(n == 0 and qc == 0),
                        stop=(n == MT - 1 and qc == 3),
                    )

        # ---- normalize + PE-transpose to feature-major oT ----
        # (not DMA transpose: Tile serializes DmaTransposeAnt with
        # collectives, which would stall the pipeline behind every RS)
        def emit_norm(st):
            j, s, po = st["j"], st["s"], st["po"]
            pst = psum.tile([P, 512], BF16, tag="ps_a", bufs=2, name="ps_t")
            for qc in range(4):
                ot = pb.tile([P, P], BF16, tag="otok", bufs=4, name="otok")
                for i in range(2):
                    rec = pb.tile([P, 1], F32, tag="rec", bufs=4, name="rec")
                    nc.vector.reciprocal(rec[:],
                                         po[i][:, qc * 65 + 64:qc * 65 + 65])
                    nc.vector.tensor_scalar_mul(
                        ot[:, i * 64:(i + 1) * 64],
                        po[i][:, qc * 65:qc * 65 + 64], rec[:])
                nc.tensor.matmul(pst[:, qc * P:(qc + 1) * P], lhsT=ot[:],
                                 rhs=ident_sb[:], is_transpose=True,
                                 start=(qc == 0), stop=(qc == 3))
            nc.vector.tensor_copy(oT[j][:, s * 512:(s + 1) * 512], pst[:])

        # ---- partial projection for token quarter q (both pairs) ----
        # bias/4 folds into the PSUM evacuation; the RS sum restores it.
        def emit_proj(q, lo, hi):
            for of in range(lo, hi):
                ps = psum.tile([P, QS], F32, tag="ps_a", bufs=2, name="ps_y")
                for j in range(2):
                    nc.tensor.matmul(
                        ps[:],
                        lhsT=wpr_sb[j][:, of * P:(of + 1) * P],
                        rhs=oT[j][:, q * QS:(q + 1) * QS],
                        start=(j == 0), stop=(j == 1),
                    )
                ysb = pb.tile([P, QS], BF16, tag="ysb", bufs=3, name="ysb")
                nc.vector.tensor_scalar_add(ysb[:], ps[:], bias_sb[of][:])
                nc.sync.dma_start(cc_in[q].ap()[of * P:(of + 1) * P, :],
                                  ysb[:])

        def emit_rs(q):
            # RS leaves this core's 256 out-columns; one DRAM->DRAM DMA
            # forwards the contiguous block to the IO tensor.
            if os.environ.get("KMODE") == "nocc":
                nc.gpsimd.dma_start(cc_out[q].ap()[:, :],
                                    cc_in[q].ap()[0:FEAT, :])
            else:
                nc.gpsimd.collective_compute(
                    "ReduceScatter",
                    mybir.AluOpType.add,
                    ins=[cc_in[q].ap()[:, :]],
                    outs=[cc_out[q].ap()[:, :]],
                    replica_groups=GROUPS,
                )

        def emit_out(q):
            nc.sync.dma_start(out_e[q, :, :], cc_out[q].ap()[:, :])

        # ---- head: strip (0,0) prerequisites ----
        emit_qk(0, names=("wk",), chunks=(0,))
        emit_qk(0, names=("wq",), chunks=(0,))

        # ---- strip pipeline ----
        # AV chunks run in-strip, LAG tiles behind the exp stream; the
        # last LAG chunks and the normalize drain right after the strip.
        # Hooks gap-fill the PE just-in-time: the first strip carries all
        # of v (AV(0,0) consumes it chunk by chunk) and its own kT0
        # chunks; later strips carry the next strip's qkv prerequisites
        # and the per-quarter partial projection + ReduceScatter.
        LAG = 2
        hooks = {
            0: {1: lambda: emit_v(0, 2), 2: lambda: emit_qk(0, ("wk",), (1,)),
                3: lambda: emit_v(2, 4), 5: lambda: emit_v(4, 6),
                6: lambda: emit_qk(0, ("wk",), (2,)),
                7: lambda: emit_v(6, 8), 9: lambda: emit_v(8, 10),
                10: lambda: emit_qk(0, ("wk",), (3,)),
                11: lambda: emit_v(10, 12), 13: lambda: emit_v(12, 14),
                14: lambda: emit_qk(0, ("wq",), (1,)),
                15: lambda: emit_v(14, 16)},
            1: {3: lambda: emit_qk(1, ("wk",), (0,)),
                7: lambda: emit_qk(1, ("wk",), (1,)),
                11: lambda: emit_qk(1, ("wq",), (0,))},
            2: {5: lambda: emit_qk(1, ("wk",), (2,)),
                9: lambda: emit_qk(1, ("wk",), (3,)),
                13: lambda: emit_qk(1, ("wq",), (1,))},
            3: {4: lambda: emit_proj(0, 0, 4), 6: lambda: emit_proj(0, 4, 8),
                8: lambda: emit_rs(0),
                10: lambda: emit_qk(0, ("wq",), (2,))},
            4: {4: lambda: emit_proj(1, 0, 4), 6: lambda: emit_proj(1, 4, 8),
                8: lambda: emit_rs(1),
                10: lambda: emit_qk(1, ("wq",), (2,))},
            5: {5: lambda: emit_qk(0, ("wq",), (3,)),
                12: lambda: emit_out(0)},
            6: {4: lambda: emit_proj(2, 0, 4), 6: lambda: emit_proj(2, 4, 8),
                8: lambda: emit_rs(2),
                10: lambda: emit_qk(1, ("wq",), (3,))},
            7: {6: lambda: emit_out(1)},
        }

        strips = [(0, 0), (0, 1), (1, 0), (1, 1),
                  (0, 2), (1, 2), (0, 3), (1, 3)]
        pending_norm = None
        for idx, (j, s) in enumerate(strips):
            cur = {"j": j, "s": s, "pt": [], "po": None}
            strip_hooks = hooks.get(idx, {})
            for n in range(MT):
                cur["pt"].append(emit_s_tile(j, s, n))
                if n == 1 and pending_norm is not None:
                    # norm of the previous strip, after this strip's first
                    # S tiles so the PE never idles at the boundary
                    emit_norm(pending_norm)
                    pending_norm = None
                if n >= LAG:
                    emit_av_chunk(cur, n - LAG)
                if n in strip_hooks:
                    strip_hooks[n]()
            for n in range(MT - LAG, MT):
                emit_av_chunk(cur, n)
            pending_norm = cur

        # ---- tail: last norm, quarter 3 ----
        emit_norm(pending_norm)
        emit_proj(3, 0, 8)
        emit_rs(3)
        emit_out(2)
        emit_out(3)

    return nc


_CACHE = {}


def _get_nc():
    if "nc" not in _CACHE:
        nc = bacc.Bacc("TRN2", target_bir_lowering=False, debug=False,
                       num_devices=N_CORES)
        nc = build_program(nc)
        nc.compile()
        _CACHE["nc"] = nc
    return _CACHE["nc"]


def make_in_maps(x, w_qkv, w_proj, b_proj):
    in_maps = []
    for core in range(N_CORES):
        b, g = core // 4, core % 4
        hs = slice(g * FEAT, (g + 1) * FEAT)
        in_maps.append({
            "xT": np.ascontiguousarray(x[b].T).astype(NPBF16),
            "wq": np.ascontiguousarray(w_qkv[:, 0:1024][:, hs]).astype(NPBF16),
            "wk": np.ascontiguousarray(w_qkv[:, 1024:2048][:, hs]).astype(NPBF16),
            "wv": np.ascontiguousarray(w_qkv[:, 2048:3072][:, hs]).astype(NPBF16),
            "wpr": np.ascontiguousarray(w_proj[hs, :]).astype(NPBF16),
            "bpf": (np.asarray(b_proj).reshape(C, 1) * 0.25).astype(np.float32),
            "ident": np.eye(P, dtype=NPBF16),
        })
    return in_maps


def assemble(results):
    out = np.empty((2, NTOK, 1024), np.float32)
    for core in range(N_CORES):
        b, g = core // 4, core % 4
        y = results[core]["out"].astype(np.float32)   # [4, FEAT, QS]
        out[b][:, g * FEAT:(g + 1) * FEAT] = \
            y.transpose(0, 2, 1).reshape(NTOK, FEAT)
    return out


def kernel(x, w_qkv, w_proj, b_proj, trace=False):
    nc = _get_nc()
    in_maps = make_in_maps(np.asarray(x), np.asarray(w_qkv),
                           np.asarray(w_proj), np.asarray(b_proj))
    res = run_bass_kernel_spmd(nc, in_maps, core_ids=list(range(N_CORES)),
                               trace=trace)
    out = assemble(res.results)
    if trace:
        return out, res
    return out


# revision 45
# speedup vs baseline: 1.3141x; 1.1572x over previous
"""Distributed attention kernel for Trainium2 (8 NeuronCores).

Module: x @ w_qkv -> per-head softmax(q k^T / sqrt(hd)) @ v -> out @ w_proj + b.
Shapes: B=2, N=2048, DIM=1024, H=16, HD=64, f32 in/out; bf16 matmul compute
(f32 PSUM accumulation).

Sharding: core i handles batch b=i//4 and head-group g=i%4 (4 heads).
- qT/kT computed feature-major [256, 2048]; v token-major with a fused
  ones column per head (65 cols) so the AV matmul also produces softmax
  denominators.
- Attention strips alternate head pairs ((0,s) then (1,s) per query
  strip s) so both pairs' outputs for a token range complete together.
  S^T matmuls for the two heads of a pair pack side by side in one
  [128, 1024] PSUM tile so a single Exp covers both heads (logits are
  O(1): no max subtraction needed).
- AV is P-stationary: out = O [128 queries, 65] per (head, q-chunk), so
  each accumulation chunk costs only 65 output rows instead of 512.  AV
  chunks pipeline one strip behind S/exp (chunk n of strip t interleaves
  after S tile n+2 of strip t+1) so PE chases the exp stream.
- Softmax normalize: row 64 of O is the denominator -> DVE reciprocal +
  per-partition tensor_scalar_mul; the token-major [128, 128] (2-head)
  result flips to feature-major oT via DMA transpose (xbar).
- Exchange: per token quarter, each core computes its partial projection
  y^T[1024 out-cols, 512 tokens] (contraction over its own 256 features
  only) and a ReduceScatter over the 4-core batch group sums partials
  and leaves each core its 256 out-columns.  Quarters 0-2 overlap the
  remaining attention strips; only quarter 3's RS is on the tail.
- Bias is a per-partition scalar add on the feature-major result.
Host assembles the 8 per-core [256, 2048] outputs (transposed) into
[2, 2048, 1024].
"""

import sys, os

for _p in ("/opt/trn_rl_repo", "/opt/pypackages"):
    if _p not in sys.path:
        sys.path.insert(0, _p)

import numpy as np
import ml_dtypes
from contextlib import ExitStack

import concourse.bass as bass
import concourse.bacc as bacc
import concourse.mybir as mybir
from concourse import tile
from concourse.bass_utils import run_bass_kernel_spmd

F32 = mybir.dt.float32
BF16 = mybir.dt.bfloat16
NPBF16 = np.dtype(ml_dtypes.bfloat16)

P = 128
NTOK = 2048
C = 1024
NH = 4          # heads per core
HD = 64
FEAT = NH * HD  # 256
KT = C // P     # 8 contraction tiles for qkv
MT = NTOK // P  # 16 key tiles per strip
QS = 512        # queries per strip
SCALE = HD ** -0.5
N_CORES = 8
GROUPS = [[0, 1, 2, 3], [4, 5, 6, 7]]

AF = mybir.ActivationFunctionType


def build_program(nc):
    xT = nc.dram_tensor("xT", [C, NTOK], BF16, kind="ExternalInput").ap()
    wq = nc.dram_tensor("wq", [C, FEAT], BF16, kind="ExternalInput").ap()
    wk = nc.dram_tensor("wk", [C, FEAT], BF16, kind="ExternalInput").ap()
    wv = nc.dram_tensor("wv", [C, FEAT], BF16, kind="ExternalInput").ap()
    # w_proj ROW slice for this core's features: [FEAT, C]
    wpr = nc.dram_tensor("wpr", [FEAT, C], BF16, kind="ExternalInput").ap()
    # full projection bias (pre-scaled by 1/4 on host: the ReduceScatter
    # sums it back across the 4-core group), added into the partials
    bpf = nc.dram_tensor("bpf", [C, 1], F32, kind="ExternalInput").ap()
    ident = nc.dram_tensor("ident", [P, P], BF16, kind="ExternalInput").ap()
    # out is per-quarter feature-major [4, FEAT, QS] bf16, each quarter a
    # contiguous block written directly by its ReduceScatter; the host
    # reassembles and upcasts
    out_e = nc.dram_tensor("out", [4, FEAT, QS], BF16, kind="ExternalOutput").ap()

    # Internal DRAM for the per-quarter ReduceScatter.
    cc_in = [nc.dram_tensor(f"cc_in{q}", [C, QS], BF16) for q in range(4)]
    cc_out = [nc.dram_tensor(f"cc_out{q}", [FEAT, QS], BF16) for q in range(4)]

    with tile.TileContext(nc) as tc, ExitStack() as ctx:
        persist = ctx.enter_context(tc.tile_pool(name="persist", bufs=1))
        psum = ctx.enter_context(tc.tile_pool(name="psum", bufs=1, space="PSUM"))

        bias_sb = [persist.tile([P, 1], F32, tag=f"bias{i}", name=f"bias{i}")
                   for i in range(KT)]
        for i in range(KT):
            nc.sync.dma_start(bias_sb[i][:], bpf[i * P:(i + 1) * P, :])
        ident_sb = persist.tile([P, P], BF16, tag="ident")
        nc.sync.dma_start(ident_sb[:], ident[:])

        # ---- persistent activations ----
        qT = [persist.tile([P, NTOK], BF16, tag=f"qT{j}", name=f"qT{j}")
              for j in range(2)]
        kT = [persist.tile([P, NTOK], BF16, tag=f"kT{j}", name=f"kT{j}")
              for j in range(2)]
        v_sb = [persist.tile([P, NH * 65], BF16, tag=f"v{m}", name=f"v{m}")
                for m in range(MT)]
        oT = [persist.tile([P, NTOK], BF16, tag=f"oT{j}", name=f"oT{j}")
              for j in range(2)]

        pa = ctx.enter_context(tc.tile_pool(name="stage_a", bufs=1))
        pb = ctx.enter_context(tc.tile_pool(name="stage_b", bufs=1))

        # ---- input loads ----
        x_sb = [pa.tile([P, NTOK], BF16, tag=f"x{k}", name=f"x{k}")
                for k in range(KT)]
        w_sb = {}
        for name, ap in (("wq", wq), ("wk", wk), ("wv", wv)):
            w_sb[name] = [pa.tile([P, FEAT], BF16, tag=f"{name}{k}",
                                  name=f"{name}{k}") for k in range(KT)]
        wpr_sb = [pa.tile([P, C], BF16, tag=f"wpr{j}", name=f"wpr{j}")
                  for j in range(2)]
        # x in column-quarters: the first S tile only needs quarter 0, so
        # interleave weight loads between quarters in need order.
        def load_xq(q):
            for k in range(KT):
                nc.sync.dma_start(x_sb[k][:, q * 512:(q + 1) * 512],
                                  xT[k * P:(k + 1) * P, q * 512:(q + 1) * 512])

        # wk issues from the ACT HWDGE queue (idle before the first exp)
        # in parallel with x on SP: the sequencer's ~0.65us per-DMA issue
        # cost would otherwise gate the first matmul
        load_xq(0)
        for k in range(KT):
            nc.scalar.dma_start(w_sb["wk"][k][:], wk[k * P:(k + 1) * P, :])
        for k in range(KT):
            nc.scalar.dma_start(w_sb["wq"][k][:], wq[k * P:(k + 1) * P, :])
        load_xq(1)
        for k in range(KT):
            nc.sync.dma_start(w_sb["wv"][k][:], wv[k * P:(k + 1) * P, :])
        load_xq(2)
        load_xq(3)
        for j in range(2):
            nc.sync.dma_start(wpr_sb[j][:], wpr[j * P:(j + 1) * P, :])

        # ---- qkv emitters (PE gap fill) ----
        def emit_qk(j, names=("wq", "wk"), chunks=(0, 1, 2, 3)):
            for wname in names:
                dst = qT if wname == "wq" else kT
                for s in chunks:
                    ps = psum.tile([P, 512], F32, tag="ps_a", bufs=2,
                                   name="ps_qk")
                    for k in range(KT):
                        nc.tensor.matmul(
                            ps[:],
                            lhsT=w_sb[wname][k][:, j * P:(j + 1) * P],
                            rhs=x_sb[k][:, s * 512:(s + 1) * 512],
                            start=(k == 0), stop=(k == KT - 1),
                        )
                    nc.vector.tensor_copy(dst[j][:, s * 512:(s + 1) * 512],
                                          ps[:])

        def emit_v(lo=0, hi=MT):
            for m in range(lo, hi):
                ps = psum.tile([P, FEAT], F32, tag="ps_a", bufs=2,
                               padded_shape=[P, 512], name="ps_v")
                for k in range(KT):
                    nc.tensor.matmul(
                        ps[:],
                        lhsT=x_sb[k][:, m * P:(m + 1) * P],
                        rhs=w_sb["wv"][k][:],
                        start=(k == 0), stop=(k == KT - 1),
                    )
                nc.gpsimd.memset(v_sb[m][:], 1.0)
                dst = v_sb[m][:].rearrange("p (h e) -> p h e", e=65)[:, :, 0:64]
                src = ps[:].rearrange("p (h e) -> p h e", e=64)
                nc.vector.tensor_copy(dst, src)

        # ---- S + exp for one (strip, key-tile) ----
        def emit_s_tile(j, s, n):
            ps = psum.tile([P, 1024], F32, tag="ps_s", bufs=2, name="ps_s")
            for i in range(2):      # head 2j at cols 0:512, 2j+1 after
                po = i * 64
                nc.tensor.matmul(
                    ps[:, i * 512:(i + 1) * 512],
                    lhsT=kT[j][po:po + 64, n * P:(n + 1) * P],
                    rhs=qT[j][po:po + 64, s * 512:(s + 1) * 512],
                    start=True, stop=True,
                )
            pt = pb.tile([P, 1024], BF16, tag="pt", bufs=20, name="pt")
            nc.scalar.activation(pt[:], ps[:], AF.Exp, scale=SCALE)
            return pt

        # ---- AV chunk: P-stationary, out O [128 q, 65] per (head, qc) ----
        def emit_av_chunk(st, n):
            j, pts = st["j"], st["pt"]
            if st["po"] is None:
                st["po"] = [psum.tile([P, 512], F32, tag="ps_o", bufs=2,
                                      name=f"po{j}{st['s']}{i}")
                            for i in range(2)]
            # one accumulation group per head's PSUM bank: only the very
            # first matmul starts it, only the very last stops it (PSUM
            # allows a single pending group per zero region)
            for i in range(2):
                h = 2 * j + i
                for qc in range(4):
                    nc.tensor.matmul(
                        st["po"][i][:, qc * 65:qc * 65 + 65],
                        lhsT=pts[n][:, i * 512 + qc * P:i * 512 + (qc + 1) * P],
                        rhs=v_sb[n][:, h * 65:(h + 1) * 65],
                        start=(n == 0 and qc == 0),
                        stop=(n == MT - 1 and qc == 3),
                    )

        # ---- normalize + PE-transpose to feature-major oT ----
        # (not DMA transpose: Tile serializes DmaTransposeAnt with
        # collectives, which would stall the pipeline behind every RS)
        def emit_norm(st):
            j, s, po = st["j"], st["s"], st["po"]
            pst = psum.tile([P, 512], BF16, tag="ps_a", bufs=2, name="ps_t")
            for qc in range(4):
                ot = pb.tile([P, P], BF16, tag="otok", bufs=4, name="otok")
                for i in range(2):
                    rec = pb.tile([P, 1], F32, tag="rec", bufs=4, name="rec")
                    nc.vector.reciprocal(rec[:],
                                         po[i][:, qc * 65 + 64:qc * 65 + 65])
                    nc.vector.tensor_scalar_mul(
                        ot[:, i * 64:(i + 1) * 64],
                        po[i][:, qc * 65:qc * 65 + 64], rec[:])
                nc.tensor.matmul(pst[:, qc * P:(qc + 1) * P], lhsT=ot[:],
                                 rhs=ident_sb[:], is_transpose=True,
                                 start=(qc == 0), stop=(qc == 3))
            nc.vector.tensor_copy(oT[j][:, s * 512:(s + 1) * 512], pst[:])

        # ---- partial projection for token quarter q (both pairs) ----
        # bias/4 folds into the PSUM evacuation; the RS sum restores it.
        def emit_proj(q, lo, hi):
            # 4 of-chunks share one wide ysb tile -> a single cc_in DMA,
            # so the RS's wait covers 2 DMAs per quarter instead of 8
            ysb = pb.tile([P, 4 * QS], BF16, tag="ysb", bufs=2, name="ysb")
            for oi, of in enumerate(range(lo, hi)):
                ps = psum.tile([P, QS], F32, tag="ps_a", bufs=2, name="ps_y")
                for j in range(2):
                    nc.tensor.matmul(
                        ps[:],
                        lhsT=wpr_sb[j][:, of * P:(of + 1) * P],
                        rhs=oT[j][:, q * QS:(q + 1) * QS],
                        start=(j == 0), stop=(j == 1),
                    )
                nc.vector.tensor_scalar_add(ysb[:, oi * QS:(oi + 1) * QS],
                                            ps[:], bias_sb[of][:])
            for h in range(2):
                dst = cc_in[q].ap()[(lo + 2 * h) * P:(lo + 2 * h + 2) * P,
                                    :].rearrange("(o p) c -> p o c", p=P)
                src = ysb[:, 2 * h * QS:(2 * h + 2) * QS]
                nc.sync.dma_start(dst,
                                  src.rearrange("p (o c) -> p o c", c=QS))

        def emit_rs(q):
            # RS leaves this core's 256 out-columns; one DRAM->DRAM DMA
            # forwards the contiguous block to the IO tensor.
            if os.environ.get("KMODE") == "nocc":
                nc.gpsimd.dma_start(cc_out[q].ap()[:, :],
                                    cc_in[q].ap()[0:FEAT, :])
            else:
                nc.gpsimd.collective_compute(
                    "ReduceScatter",
                    mybir.AluOpType.add,
                    ins=[cc_in[q].ap()[:, :]],
                    outs=[cc_out[q].ap()[:, :]],
                    replica_groups=GROUPS,
                )

        def emit_out(q, eng=None):
            # tail outs ride the ACT queue (idle after the last exp); the
            # SP queue reorders and head-of-line blocks behind RS waits.
            # Row-interleaved halves keep the lowered AP unmerged: the cost
            # model prices a DMA by free-size after the leading dim.
            e = eng or nc.scalar
            for h in range(2):
                e.dma_start(
                    out_e[q, :, :].rearrange("(a b) c -> a b c", b=2)[:, h, :],
                    cc_out[q].ap().rearrange("(a b) c -> a b c", b=2)[:, h, :])

        # ---- head: strip (0,0) prerequisites ----
        emit_qk(0, names=("wk",), chunks=(0,))
        emit_qk(0, names=("wq",), chunks=(0,))

        # ---- strip pipeline ----
        # AV chunks run in-strip, LAG tiles behind the exp stream; the
        # last LAG chunks and the normalize drain right after the strip.
        # Hooks gap-fill the PE just-in-time: the first strip carries all
        # of v (AV(0,0) consumes it chunk by chunk) and its own kT0
        # chunks; later strips carry the next strip's qkv prerequisites
        # and the per-quarter partial projection + ReduceScatter.
        LAG = 2
        hooks = {
            0: {1: lambda: emit_v(0, 2), 2: lambda: emit_qk(0, ("wk",), (1,)),
                3: lambda: emit_v(2, 4), 5: lambda: emit_v(4, 6),
                6: lambda: emit_qk(0, ("wk",), (2,)),
                7: lambda: emit_v(6, 8), 9: lambda: emit_v(8, 10),
                10: lambda: emit_qk(0, ("wk",), (3,)),
                11: lambda: emit_v(10, 12), 13: lambda: emit_v(12, 14),
                14: lambda: emit_qk(0, ("wq",), (1,)),
                15: lambda: emit_v(14, 16)},
            1: {3: lambda: emit_qk(1, ("wk",), (0,)),
                7: lambda: emit_qk(1, ("wk",), (1,)),
                11: lambda: emit_qk(1, ("wq",), (0,))},
            2: {5: lambda: emit_qk(1, ("wk",), (2,)),
                9: lambda: emit_qk(1, ("wk",), (3,)),
                13: lambda: emit_qk(1, ("wq",), (1,))},
            3: {4: lambda: emit_proj(0, 0, 4), 6: lambda: emit_proj(0, 4, 8),
                8: lambda: emit_rs(0),
                10: lambda: emit_qk(0, ("wq",), (2,))},
            4: {4: lambda: emit_proj(1, 0, 4), 6: lambda: emit_proj(1, 4, 8),
                8: lambda: emit_rs(1),
                10: lambda: emit_qk(1, ("wq",), (2,))},
            5: {5: lambda: emit_qk(0, ("wq",), (3,))},
            6: {4: lambda: emit_proj(2, 0, 4), 6: lambda: emit_proj(2, 4, 8),
                8: lambda: emit_rs(2),
                10: lambda: emit_qk(1, ("wq",), (3,))},
            7: {2: lambda: emit_out(0, nc.sync)},
        }

        strips = [(0, 0), (0, 1), (1, 0), (1, 1),
                  (0, 2), (1, 2), (0, 3), (1, 3)]
        pending_norm = None
        for idx, (j, s) in enumerate(strips):
            cur = {"j": j, "s": s, "pt": [], "po": None}
            strip_hooks = hooks.get(idx, {})
            for n in range(MT):
                cur["pt"].append(emit_s_tile(j, s, n))
                if n == 1 and pending_norm is not None:
                    # norm of the previous strip, after this strip's first
                    # S tiles so the PE never idles at the boundary
                    emit_norm(pending_norm)
                    pending_norm = None
                if n >= LAG:
                    emit_av_chunk(cur, n - LAG)
                if n in strip_hooks:
                    strip_hooks[n]()
            for n in range(MT - LAG, MT):
                emit_av_chunk(cur, n)
            pending_norm = cur

        # ---- tail: last norm, quarter 3 ----
        emit_norm(pending_norm)
        emit_out(1)
        emit_proj(3, 0, 4)
        emit_proj(3, 4, 8)
        emit_rs(3)
        emit_out(2)
        emit_out(3)

    return nc


_CACHE = {}


def _get_nc():
    if "nc" not in _CACHE:
        nc = bacc.Bacc("TRN2", target_bir_lowering=False, debug=False,
                       num_devices=N_CORES)
        nc = build_program(nc)
        nc.compile()
        _CACHE["nc"] = nc
    return _CACHE["nc"]


def make_in_maps(x, w_qkv, w_proj, b_proj):
    in_maps = []
    for core in range(N_CORES):
        b, g = core // 4, core % 4
        hs = slice(g * FEAT, (g + 1) * FEAT)
        in_maps.append({
            "xT": np.ascontiguousarray(x[b].T).astype(NPBF16),
            "wq": np.ascontiguousarray(w_qkv[:, 0:1024][:, hs]).astype(NPBF16),
            "wk": np.ascontiguousarray(w_qkv[:, 1024:2048][:, hs]).astype(NPBF16),
            "wv": np.ascontiguousarray(w_qkv[:, 2048:3072][:, hs]).astype(NPBF16),
            "wpr": np.ascontiguousarray(w_proj[hs, :]).astype(NPBF16),
            "bpf": (np.asarray(b_proj).reshape(C, 1) * 0.25).astype(np.float32),
            "ident": np.eye(P, dtype=NPBF16),
        })
    return in_maps


def assemble(results):
    out = np.empty((2, NTOK, 1024), np.float32)
    for core in range(N_CORES):
        b, g = core // 4, core % 4
        y = results[core]["out"].astype(np.float32)   # [4, FEAT, QS]
        out[b][:, g * FEAT:(g + 1) * FEAT] = \
            y.transpose(0, 2, 1).reshape(NTOK, FEAT)
    return out


def kernel(x, w_qkv, w_proj, b_proj, trace=False):
    nc = _get_nc()
    in_maps = make_in_maps(np.asarray(x), np.asarray(w_qkv),
                           np.asarray(w_proj), np.asarray(b_proj))
    res = run_bass_kernel_spmd(nc, in_maps, core_ids=list(range(N_CORES)),
                               trace=trace)
    out = assemble(res.results)
    if trace:
        return out, res
    return out


# revision 58
# speedup vs baseline: 1.3459x; 1.0242x over previous
"""Distributed attention kernel for Trainium2 (8 NeuronCores).

Module: x @ w_qkv -> per-head softmax(q k^T / sqrt(hd)) @ v -> out @ w_proj + b.
Shapes: B=2, N=2048, DIM=1024, H=16, HD=64, f32 in/out; bf16 matmul compute
(f32 PSUM accumulation).

Sharding: core i handles batch b=i//4 and head-group g=i%4 (4 heads).
- qT/kT computed feature-major [256, 2048]; v token-major with a fused
  ones column per head (65 cols) so the AV matmul also produces softmax
  denominators.
- Attention strips alternate head pairs ((0,s) then (1,s) per query
  strip s) so both pairs' outputs for a token range complete together.
  S^T matmuls for the two heads of a pair pack side by side in one
  [128, 1024] PSUM tile so a single Exp covers both heads (logits are
  O(1): no max subtraction needed).
- AV is P-stationary: out = O [128 queries, 65] per (head, q-chunk), so
  each accumulation chunk costs only 65 output rows instead of 512.  AV
  chunks pipeline one strip behind S/exp (chunk n of strip t interleaves
  after S tile n+2 of strip t+1) so PE chases the exp stream.
- Softmax normalize: row 64 of O is the denominator -> DVE reciprocal +
  per-partition tensor_scalar_mul; the token-major [128, 128] (2-head)
  result flips to feature-major oT via DMA transpose (xbar).
- Exchange: per token quarter, each core computes its partial projection
  y^T[1024 out-cols, 512 tokens] (contraction over its own 256 features
  only) and a ReduceScatter over the 4-core batch group sums partials
  and leaves each core its 256 out-columns.  Quarters 0-2 overlap the
  remaining attention strips; only quarter 3's RS is on the tail.
- Bias is a per-partition scalar add on the feature-major result.
Host assembles the 8 per-core [256, 2048] outputs (transposed) into
[2, 2048, 1024].
"""

import sys, os

for _p in ("/opt/trn_rl_repo", "/opt/pypackages"):
    if _p not in sys.path:
        sys.path.insert(0, _p)

import numpy as np
import ml_dtypes
from contextlib import ExitStack

import concourse.bass as bass
import concourse.bacc as bacc
import concourse.mybir as mybir
from concourse import tile
from concourse.bass_utils import run_bass_kernel_spmd

F32 = mybir.dt.float32
BF16 = mybir.dt.bfloat16
NPBF16 = np.dtype(ml_dtypes.bfloat16)

P = 128
NTOK = 2048
C = 1024
NH = 4          # heads per core
HD = 64
FEAT = NH * HD  # 256
KT = C // P     # 8 contraction tiles for qkv
MT = NTOK // P  # 16 key tiles per strip
QS = 512        # queries per strip
SCALE = HD ** -0.5
N_CORES = 8
GROUPS = [[0, 1, 2, 3], [4, 5, 6, 7]]

AF = mybir.ActivationFunctionType


def build_program(nc):
    xT = nc.dram_tensor("xT", [C, NTOK], BF16, kind="ExternalInput").ap()
    wq = nc.dram_tensor("wq", [C, FEAT], BF16, kind="ExternalInput").ap()
    wk = nc.dram_tensor("wk", [C, FEAT], BF16, kind="ExternalInput").ap()
    wv = nc.dram_tensor("wv", [C, FEAT], BF16, kind="ExternalInput").ap()
    # w_proj ROW slice for this core's features: [FEAT, C]
    wpr = nc.dram_tensor("wpr", [FEAT, C], BF16, kind="ExternalInput").ap()
    # full projection bias (pre-scaled by 1/4 on host: the ReduceScatter
    # sums it back across the 4-core group), added into the partials
    bpf = nc.dram_tensor("bpf", [C, 1], F32, kind="ExternalInput").ap()
    bpr = nc.dram_tensor("bpr", [1, C], BF16, kind="ExternalInput").ap()
    ident = nc.dram_tensor("ident", [P, P], BF16, kind="ExternalInput").ap()
    # out is per-quarter feature-major [4, FEAT, QS] bf16, each quarter a
    # contiguous block written directly by its ReduceScatter; the host
    # reassembles and upcasts
    out_e = nc.dram_tensor("out", [4, FEAT, QS], BF16, kind="ExternalOutput").ap()

    # Internal DRAM for the per-quarter ReduceScatter.
    cc_in = [nc.dram_tensor(f"cc_in{q}", [C, QS], BF16) for q in range(4)]
    cc_out = [nc.dram_tensor(f"cc_out{q}", [FEAT, QS], BF16) for q in range(4)]

    with tile.TileContext(nc) as tc, ExitStack() as ctx:
        persist = ctx.enter_context(tc.tile_pool(name="persist", bufs=1))
        psum = ctx.enter_context(tc.tile_pool(name="psum", bufs=1, space="PSUM"))

        bias_sb = [persist.tile([P, 1], F32, tag=f"bias{i}", name=f"bias{i}")
                   for i in range(KT)]
        ident_sb = persist.tile([P, P], BF16, tag="ident")
        bias_row = persist.tile([1, C], BF16, tag="bias_row")
        ones_row = persist.tile([1, QS], BF16, tag="ones_row")

        # ---- persistent activations ----
        qT = [persist.tile([P, NTOK], BF16, tag=f"qT{j}", name=f"qT{j}")
              for j in range(2)]
        kT = [persist.tile([P, NTOK], BF16, tag=f"kT{j}", name=f"kT{j}")
              for j in range(2)]
        v_sb = [persist.tile([P, NH * 65], BF16, tag=f"v{m}", name=f"v{m}")
                for m in range(MT)]
        oT = [persist.tile([P, NTOK], BF16, tag=f"oT{j}", name=f"oT{j}")
              for j in range(2)]

        pa = ctx.enter_context(tc.tile_pool(name="stage_a", bufs=1))
        pb = ctx.enter_context(tc.tile_pool(name="stage_b", bufs=1))

        # ---- input loads ----
        x_sb = [pa.tile([P, NTOK], BF16, tag=f"x{k}", name=f"x{k}")
                for k in range(KT)]
        w_sb = {}
        for name, ap in (("wq", wq), ("wk", wk), ("wv", wv)):
            w_sb[name] = [pa.tile([P, FEAT], BF16, tag=f"{name}{k}",
                                  name=f"{name}{k}") for k in range(KT)]
        wpr_sb = [pa.tile([P, C], BF16, tag=f"wpr{j}", name=f"wpr{j}")
                  for j in range(2)]
        # x in column-quarters: the first S tile only needs quarter 0, so
        # interleave weight loads between quarters in need order.
        def load_xq(q):
            for k in range(KT):
                nc.sync.dma_start(x_sb[k][:, q * 512:(q + 1) * 512],
                                  xT[k * P:(k + 1) * P, q * 512:(q + 1) * 512])

        # Input loads spread over three queues (SP / ACT-HWDGE / Pool-SWDGE)
        # so the ~0.65us per-DMA sequencer issue cost doesn't gate the
        # first matmuls; everything is ordered by need.
        nc.gpsimd.dma_start(ident_sb[:], ident[:])
        load_xq(0)
        for k in range(KT):
            nc.scalar.dma_start(w_sb["wk"][k][:], wk[k * P:(k + 1) * P, :])
        for k in range(4):
            nc.scalar.dma_start(w_sb["wq"][k][:], wq[k * P:(k + 1) * P, :])
        for k in range(4, KT):
            nc.sync.dma_start(w_sb["wq"][k][:], wq[k * P:(k + 1) * P, :])
        for k in range(KT):
            nc.gpsimd.dma_start(w_sb["wv"][k][:], wv[k * P:(k + 1) * P, :])
        load_xq(1)
        load_xq(2)
        load_xq(3)
        for j in range(2):
            nc.sync.dma_start(wpr_sb[j][:], wpr[j * P:(j + 1) * P, :])
        for i in range(KT):
            nc.sync.dma_start(bias_sb[i][:], bpf[i * P:(i + 1) * P, :])
        nc.sync.dma_start(bias_row[:], bpr[:])
        nc.gpsimd.memset(ones_row[:], 1.0)

        # PE p-state warmup: garbage matmuls so the ramp completes before
        # the first qkv matmuls arrive (results never read)
        ps_w = psum.tile([P, 512], F32, tag="ps_o", bufs=2, name="ps_w")
        for _ in range(12):
            nc.tensor.matmul(ps_w[:, 0:P], lhsT=ident_sb[:], rhs=ident_sb[:],
                             start=True, stop=True)

        # ---- qkv emitters (PE gap fill) ----
        def emit_qk(j, names=("wq", "wk"), chunks=(0, 1, 2, 3)):
            for wname in names:
                dst = qT if wname == "wq" else kT
                for s in chunks:
                    ps = psum.tile([P, 512], F32, tag="ps_a", bufs=2,
                                   name="ps_qk")
                    for k in range(KT):
                        nc.tensor.matmul(
                            ps[:],
                            lhsT=w_sb[wname][k][:, j * P:(j + 1) * P],
                            rhs=x_sb[k][:, s * 512:(s + 1) * 512],
                            start=(k == 0), stop=(k == KT - 1),
                        )
                    nc.vector.tensor_copy(dst[j][:, s * 512:(s + 1) * 512],
                                          ps[:])

        def emit_v(lo=0, hi=MT):
            for m in range(lo, hi):
                ps = psum.tile([P, FEAT], F32, tag="ps_a", bufs=2,
                               padded_shape=[P, 512], name="ps_v")
                for k in range(KT):
                    nc.tensor.matmul(
                        ps[:],
                        lhsT=x_sb[k][:, m * P:(m + 1) * P],
                        rhs=w_sb["wv"][k][:],
                        start=(k == 0), stop=(k == KT - 1),
                    )
                nc.gpsimd.memset(v_sb[m][:], 1.0)
                dst = v_sb[m][:].rearrange("p (h e) -> p h e", e=65)[:, :, 0:64]
                src = ps[:].rearrange("p (h e) -> p h e", e=64)
                nc.vector.tensor_copy(dst, src)

        # ---- S + exp for one (strip, key-tile) ----
        def emit_s_tile(j, s, n):
            ps = psum.tile([P, 1024], F32, tag="ps_s", bufs=2, name="ps_s")
            for i in range(2):      # head 2j at cols 0:512, 2j+1 after
                po = i * 64
                nc.tensor.matmul(
                    ps[:, i * 512:(i + 1) * 512],
                    lhsT=kT[j][po:po + 64, n * P:(n + 1) * P],
                    rhs=qT[j][po:po + 64, s * 512:(s + 1) * 512],
                    start=True, stop=True,
                )
            pt = pb.tile([P, 1024], BF16, tag="pt", bufs=20, name="pt")
            nc.scalar.activation(pt[:], ps[:], AF.Exp, scale=SCALE)
            return pt

        # ---- AV chunk: P-stationary, out O [128 q, 65] per (head, qc) ----
        def emit_av_chunk(st, n):
            j, pts = st["j"], st["pt"]
            if st["po"] is None:
                st["po"] = [psum.tile([P, 512], F32, tag="ps_o", bufs=2,
                                      name=f"po{j}{st['s']}{i}")
                            for i in range(2)]
            # one accumulation group per head's PSUM bank: only the very
            # first matmul starts it, only the very last stops it (PSUM
            # allows a single pending group per zero region)
            for i in range(2):
                h = 2 * j + i
                for qc in range(4):
                    nc.tensor.matmul(
                        st["po"][i][:, qc * 65:qc * 65 + 65],
                        lhsT=pts[n][:, i * 512 + qc * P:i * 512 + (qc + 1) * P],
                        rhs=v_sb[n][:, h * 65:(h + 1) * 65],
                        start=(n == 0 and qc == 0),
                        stop=(n == MT - 1 and qc == 3),
                    )

        # ---- normalize + PE-transpose to feature-major oT ----
        # (not DMA transpose: Tile serializes DmaTransposeAnt with
        # collectives, which would stall the pipeline behind every RS)
        def emit_norm(st, tail=False):
            j, s, po = st["j"], st["s"], st["po"]
            pst = psum.tile([P, 512], BF16, tag="ps_a", bufs=2, name="ps_t")
            for qc in range(4):
                ot = pb.tile([P, P], BF16, tag="otok", bufs=4, name="otok")
                for i in range(2):
                    rec = pb.tile([P, 1], F32, tag="rec", bufs=4, name="rec")
                    nc.vector.reciprocal(rec[:],
                                         po[i][:, qc * 65 + 64:qc * 65 + 65])
                    if tail and i == 1:
                        # ACT is idle after the last exp: it can scale by
                        # a per-partition AP, halving the DVE chain
                        nc.scalar.activation(ot[:, 64:128],
                                             po[1][:, qc * 65:qc * 65 + 64],
                                             AF.Copy, scale=rec[:])
                    else:
                        nc.vector.tensor_scalar_mul(
                            ot[:, i * 64:(i + 1) * 64],
                            po[i][:, qc * 65:qc * 65 + 64], rec[:])
                nc.tensor.matmul(pst[:, qc * P:(qc + 1) * P], lhsT=ot[:],
                                 rhs=ident_sb[:], is_transpose=True,
                                 start=(qc == 0), stop=(qc == 3))
            nc.vector.tensor_copy(oT[j][:, s * 512:(s + 1) * 512], pst[:])

        # ---- partial projection for token quarter q (both pairs) ----
        # bias/4 folds into the PSUM evacuation; the RS sum restores it.
        def emit_proj(q, lo, hi, tail=False):
            # 4 of-chunks share one wide ysb tile -> one cc_in DMA per 2
            # chunks, so the RS's wait covers few DMAs. On the tail
            # quarter, evacuations alternate DVE/ACT (bias folded into a
            # rank-1 matmul for the ACT ones) to halve the serial chain.
            ysb = pb.tile([P, 4 * QS], BF16, tag="ysb", bufs=2, name="ysb")
            ps2 = None
            for oi, of in enumerate(range(lo, hi)):
                if tail:
                    # S/exp are done: their wider PSUM ring keeps 4
                    # of-chunks in flight so evacuations never gate matmuls
                    if oi % 2 == 0:
                        ps2 = psum.tile([P, 2 * QS], F32, tag="ps_s", bufs=2,
                                        name="ps_y2")
                    ps = ps2[:, (oi % 2) * QS:(oi % 2 + 1) * QS]
                else:
                    ps = psum.tile([P, QS], F32, tag="ps_a", bufs=2,
                                   name="ps_y")[:]
                act_evac = tail and oi % 2 == 1
                for j in range(2):
                    nc.tensor.matmul(
                        ps[:],
                        lhsT=wpr_sb[j][:, of * P:(of + 1) * P],
                        rhs=oT[j][:, q * QS:(q + 1) * QS],
                        start=(j == 0), stop=(j == 1 and not act_evac),
                    )
                if act_evac:
                    nc.tensor.matmul(ps[:], lhsT=bias_row[:, of * P:
                                                          (of + 1) * P],
                                     rhs=ones_row[:], start=False, stop=True)
                    nc.scalar.activation(ysb[:, oi * QS:(oi + 1) * QS],
                                         ps[:], AF.Copy)
                else:
                    nc.vector.tensor_scalar_add(ysb[:, oi * QS:(oi + 1) * QS],
                                                ps[:], bias_sb[of][:])
            for h in range(2):
                dst = cc_in[q].ap()[(lo + 2 * h) * P:(lo + 2 * h + 2) * P,
                                    :].rearrange("(o p) c -> p o c", p=P)
                src = ysb[:, 2 * h * QS:(2 * h + 2) * QS]
                nc.sync.dma_start(dst,
                                  src.rearrange("p (o c) -> p o c", c=QS))

        def emit_rs(q):
            # RS leaves this core's 256 out-columns; one DRAM->DRAM DMA
            # forwards the contiguous block to the IO tensor.
            if os.environ.get("KMODE") == "nocc":
                nc.gpsimd.dma_start(cc_out[q].ap()[:, :],
                                    cc_in[q].ap()[0:FEAT, :])
            else:
                nc.gpsimd.collective_compute(
                    "ReduceScatter",
                    mybir.AluOpType.add,
                    ins=[cc_in[q].ap()[:, :]],
                    outs=[cc_out[q].ap()[:, :]],
                    replica_groups=GROUPS,
                )

        def emit_out(q, eng=None):
            # tail outs ride the ACT queue (idle after the last exp); the
            # SP queue reorders and head-of-line blocks behind RS waits.
            # Row-interleaved halves keep the lowered AP unmerged: the cost
            # model prices a DMA by free-size after the leading dim.
            e = eng or nc.scalar
            for h in range(2):
                e.dma_start(
                    out_e[q, :, :].rearrange("(a b) c -> a b c", b=2)[:, h, :],
                    cc_out[q].ap().rearrange("(a b) c -> a b c", b=2)[:, h, :])

        # ---- head: strip (0,0) prerequisites ----
        emit_qk(0, names=("wk",), chunks=(0,))
        emit_qk(0, names=("wq",), chunks=(0,))

        # ---- strip pipeline ----
        # AV chunks run in-strip, LAG tiles behind the exp stream; the
        # last LAG chunks and the normalize drain right after the strip.
        # Hooks gap-fill the PE just-in-time: the first strip carries all
        # of v (AV(0,0) consumes it chunk by chunk) and its own kT0
        # chunks; later strips carry the next strip's qkv prerequisites
        # and the per-quarter partial projection + ReduceScatter.
        LAG = 2
        hooks = {
            0: {1: lambda: emit_v(0, 2), 2: lambda: emit_qk(0, ("wk",), (1,)),
                3: lambda: emit_v(2, 4), 5: lambda: emit_v(4, 6),
                6: lambda: emit_qk(0, ("wk",), (2,)),
                7: lambda: emit_v(6, 8), 9: lambda: emit_v(8, 10),
                10: lambda: emit_qk(0, ("wk",), (3,)),
                11: lambda: emit_v(10, 12), 13: lambda: emit_v(12, 14),
                14: lambda: emit_qk(0, ("wq",), (1,)),
                15: lambda: emit_v(14, 16)},
            1: {3: lambda: emit_qk(1, ("wk",), (0,)),
                7: lambda: emit_qk(1, ("wk",), (1,)),
                11: lambda: emit_qk(1, ("wq",), (0,))},
            2: {5: lambda: emit_qk(1, ("wk",), (2,)),
                9: lambda: emit_qk(1, ("wk",), (3,)),
                13: lambda: emit_qk(1, ("wq",), (1,))},
            3: {4: lambda: emit_proj(0, 0, 4), 6: lambda: emit_proj(0, 4, 8),
                8: lambda: emit_rs(0),
                10: lambda: emit_qk(0, ("wq",), (2,))},
            4: {4: lambda: emit_proj(1, 0, 4), 6: lambda: emit_proj(1, 4, 8),
                8: lambda: emit_rs(1),
                10: lambda: emit_qk(1, ("wq",), (2,))},
            5: {5: lambda: emit_qk(0, ("wq",), (3,))},
            6: {4: lambda: emit_proj(2, 0, 4), 6: lambda: emit_proj(2, 4, 8),
                8: lambda: emit_rs(2),
                10: lambda: emit_qk(1, ("wq",), (3,))},
            7: {2: lambda: emit_out(0, nc.sync)},
        }

        strips = [(0, 0), (0, 1), (1, 0), (1, 1),
                  (0, 2), (1, 2), (0, 3), (1, 3)]
        pending_norm = None
        for idx, (j, s) in enumerate(strips):
            cur = {"j": j, "s": s, "pt": [], "po": None}
            strip_hooks = hooks.get(idx, {})
            for n in range(MT):
                cur["pt"].append(emit_s_tile(j, s, n))
                if n == 1 and pending_norm is not None:
                    # norm of the previous strip, after this strip's first
                    # S tiles so the PE never idles at the boundary
                    emit_norm(pending_norm)
                    pending_norm = None
                if n >= LAG:
                    emit_av_chunk(cur, n - LAG)
                if n in strip_hooks:
                    strip_hooks[n]()
            for n in range(MT - LAG, MT):
                emit_av_chunk(cur, n)
            pending_norm = cur

        # ---- tail: last norm, quarter 3 ----
        emit_norm(pending_norm, tail=True)
        emit_out(1)
        emit_proj(3, 0, 4, tail=True)
        emit_proj(3, 4, 8, tail=True)
        emit_rs(3)
        emit_out(2)
        emit_out(3)

    return nc


_CACHE = {}


def _get_nc():
    if "nc" not in _CACHE:
        nc = bacc.Bacc("TRN2", target_bir_lowering=False, debug=False,
                       num_devices=N_CORES)
        nc = build_program(nc)
        nc.compile()
        _CACHE["nc"] = nc
    return _CACHE["nc"]


def make_in_maps(x, w_qkv, w_proj, b_proj):
    in_maps = []
    for core in range(N_CORES):
        b, g = core // 4, core % 4
        hs = slice(g * FEAT, (g + 1) * FEAT)
        in_maps.append({
            "xT": np.ascontiguousarray(x[b].T).astype(NPBF16),
            "wq": np.ascontiguousarray(w_qkv[:, 0:1024][:, hs]).astype(NPBF16),
            "wk": np.ascontiguousarray(w_qkv[:, 1024:2048][:, hs]).astype(NPBF16),
            "wv": np.ascontiguousarray(w_qkv[:, 2048:3072][:, hs]).astype(NPBF16),
            "wpr": np.ascontiguousarray(w_proj[hs, :]).astype(NPBF16),
            "bpf": (np.asarray(b_proj).reshape(C, 1) * 0.25).astype(np.float32),
            "bpr": (np.asarray(b_proj).reshape(1, C) * 0.25).astype(NPBF16),
            "ident": np.eye(P, dtype=NPBF16),
        })
    return in_maps


def assemble(results):
    out = np.empty((2, NTOK, 1024), np.float32)
    for core in range(N_CORES):
        b, g = core // 4, core % 4
        y = results[core]["out"].astype(np.float32)   # [4, FEAT, QS]
        out[b][:, g * FEAT:(g + 1) * FEAT] = \
            y.transpose(0, 2, 1).reshape(NTOK, FEAT)
    return out


def kernel(x, w_qkv, w_proj, b_proj, trace=False):
    nc = _get_nc()
    in_maps = make_in_maps(np.asarray(x), np.asarray(w_qkv),
                           np.asarray(w_proj), np.asarray(b_proj))
    res = run_bass_kernel_spmd(nc, in_maps, core_ids=list(range(N_CORES)),
                               trace=trace)
    out = assemble(res.results)
    if trace:
        return out, res
    return out
